# revision 62
# baseline (speedup 1.0000x reference)
"""Trainium2 Bass kernel: ViT-style global attention with decomposed
relative position bias (B=8, 32x32 tokens, dim 768, 12 heads, hd 64).

Sharding: data-parallel over batch B=8 -> one image per NeuronCore,
weights replicated, no collectives. All matmul operands bf16 (fp32
PSUM accumulate); TimelineSim ~160us/core vs 206us baseline.

Fully software-pipelined single pass over 6 head-pairs (emission order
defines each engine's program order; Tile adds the semaphores):
  - production(P) [bf16 chains over XT]: q/k features feature-major,
    rel-pos per pair via a shared block-diagonal stationary (the rel-h
    and rel-w placements occupy disjoint columns; garbage rows are
    never drained), V token-major. Emitted as PE "fillers" interleaved
    between the previous pair's QK matmuls with a front-loaded budget
    (q/rel/k complete during the even head so the next pair's QK is
    never production-gated).
  - attention(head): per kb ONE K=128 matmul gives scale*S^T+bias in
    PSUM (bias rides contraction rows 64:128 against indicator rows of
    K'); exp on ACT -> PT bf16. PV is token-major: lhsT = PT 128x128
    chunk (stationary), rhs = V[keys,65] (64 v-feats | ones) ->
    out[tok,65] accumulated over kb in two 4-token-block PSUM tiles;
    col 64 = softmax denominator for free. Drain fuses the divide: one
    DVE reciprocal per half + one stride-0-broadcast multiply into
    token-major aodt.
  - cleanup(pair): PE transposes (identity matmul) back to
    feature-major AOD bf16 for proj, deferred half a step so the DVE
    chain clears first.
  - proj: c=0..3 partial contractions run as late-step fillers into an
    SBUF accumulator YA (they only need AOD pairs 0..3); the tail is
    just c=4..5 plus a scalar_tensor_tensor drain (ps+bias+YA). v-bias
    is folded into proj_b on the host (y is affine in v); q-scale and
    rel-table scale are host-folded.
Engine totals: PE ~134us (bound), ACT ~108us (exp-dominated), DVE
~109us (all PSUM drains; GPSIMD cannot touch PSUM on TRN2), Pool idle.
Prologue rel tiles borrow the idle attention "s" PSUM slots.
"""

import os

import numpy as np

import concourse.bacc as bacc
import concourse.bass as bass
import concourse.tile as tile
from concourse import mybir
from concourse import bass_utils

B, H, W, DIM = 8, 32, 32, 768
HEADS, HD = 12, 64
N = H * W  # 1024
NCORES = 8
SCALE = HD ** -0.5
F32 = mybir.dt.float32
F32R = mybir.dt.float32r
BF16 = mybir.dt.bfloat16
EXP = mybir.ActivationFunctionType.Exp
IDN = mybir.ActivationFunctionType.Identity
ADD = mybir.AluOpType.add
MULT = mybir.AluOpType.mult

NC = DIM // 128      # 6 feature chunks == head pairs
NKB = N // 128       # 8 key/token blocks
NQH = N // 512       # 2 query halves

_CACHE = {}

NWARM = int(os.environ.get("K_WARM", "30"))
PT_BUFS = int(os.environ.get("K_PT", "16"))


def build_nc():
    nc = bacc.Bacc("TRN2", target_bir_lowering=False, debug=False)

    xT = nc.dram_tensor("xT", (DIM, N), BF16, kind="ExternalInput").ap()
    # packed per-pair weight columns: (pair, DIM, 384) = (q_p | k_p | v_p)
    wpack = nc.dram_tensor("wpack", (NC, DIM, 384), BF16, kind="ExternalInput").ap()
    qkvb = nc.dram_tensor("qkvb", (3 * DIM,), F32, kind="ExternalInput").ap()
    wprojT = nc.dram_tensor("wprojT", (DIM, DIM), BF16, kind="ExternalInput").ap()
    projb = nc.dram_tensor("projb", (DIM,), F32, kind="ExternalInput").ap()
    # bdfull: host-packed block-diagonal rel stationary [128, H, 128]
    bdfull = nc.dram_tensor("bdfull", (128, H, 128), BF16, kind="ExternalInput").ap()
    kconst = nc.dram_tensor("kconst", (64, N), BF16, kind="ExternalInput").ap()
    ident = nc.dram_tensor("ident", (128, 128), BF16, kind="ExternalInput").ap()
    y = nc.dram_tensor("y", (DIM, N), F32, kind="ExternalOutput").ap()

    qkvb2 = qkvb.rearrange("(c p one) -> c p one", p=128, one=1)   # [18][128,1]

    with tile.TileContext(nc) as tc:
        # ---- long-lived pools (bottom of SBUF stack) ----
        qall_p = tc.alloc_tile_pool(name="qall", bufs=1)
        kall_p = tc.alloc_tile_pool(name="kall", bufs=1)
        vall_p = tc.alloc_tile_pool(name="vall", bufs=1)
        cons_p = tc.alloc_tile_pool(name="cons", bufs=1)
        xt_p = tc.alloc_tile_pool(name="xtp", bufs=1)
        aod_p = tc.alloc_tile_pool(name="aod", bufs=1)

        QALL = qall_p.tile([128, HEADS, N], BF16)
        KALL = kall_p.tile([128, HEADS, N], BF16)
        VALL = vall_p.tile([128, HEADS, NKB, 65], BF16)
        STQB = cons_p.tile([128, NC, N], BF16)
        BD = cons_p.tile([128, H, 128], BF16)
        IDT = cons_p.tile([128, 128], BF16)
        PBIAS = cons_p.tile([128, NC], F32)
        YA = cons_p.tile([128, NC, N], F32)   # proj partial (c=0..2)
        XT = xt_p.tile([128, NC, N], BF16, tag="xtslot")
        AOD = aod_p.tile([128, NC, N], BF16)

        with tc.tile_pool(name="wpool", bufs=1) as w_p, \
             tc.tile_pool(name="bias", bufs=4) as b_p, \
             tc.tile_pool(name="pt", bufs=PT_BUFS) as pt_p, \
             tc.tile_pool(name="stg", bufs=2) as stg_p, \
             tc.tile_pool(name="psA", bufs=2, space="PSUM") as psA_p, \
             tc.tile_pool(name="psB", bufs=2, space="PSUM") as psB_p:

            # ---------- constant / early DMAs ----------
            wts = {}

            def dma_pair_weights(p):
                wt = w_p.tile([128, NC, 384], BF16, tag="wt", bufs=2,
                              name=f"wt{p}")
                nc.sync.dma_start(
                    out=wt, in_=wpack[p].rearrange("(c p2) f -> p2 c f", p2=128))
                wts[p] = wt

            def dma_pair_biases(p):
                qb = b_p.tile([128, 1], F32, tag="bias", bufs=6, name=f"qb{p}")
                nc.sync.dma_start(out=qb, in_=qkvb2[p])
                kb2 = b_p.tile([128, 1], F32, tag="bias", bufs=6, name=f"kb{p}")
                nc.sync.dma_start(out=kb2, in_=qkvb2[6 + p])
                return qb, kb2

            def dma_pair_kconst(p):
                nc.sync.dma_start(out=KALL[64:128, 2 * p, :], in_=kconst)
                nc.sync.dma_start(out=KALL[0:64, 2 * p + 1, :], in_=kconst)

            # PE p-state warm-up while DMAs land (memset first so DVE's later
            # big memsets don't gate the first warm matmul)
            if NWARM:
                jnk = cons_p.tile([128, 128], BF16)
                nc.vector.memset(jnk, 0.5)
                # preload the Exp table while the PE warms up so the first
                # real exp doesn't eat the 1.3us table load
                nc.scalar.activation(jnk[:, 0:8], jnk[:, 0:8], EXP)
                wps = psA_p.tile([128, 128], F32, tag="ps1", name="warmps")
                for _ in range(NWARM):
                    nc.tensor.matmul(wps, lhsT=jnk, rhs=jnk,
                                     start=True, stop=True,
                                     skip_group_check=True)

            dma_pair_weights(0)
            biases = {0: dma_pair_biases(0)}
            biases[1] = dma_pair_biases(1)
            xTr = xT.rearrange("(c p2) n -> p2 c n", p2=128)
            for c in range(NC):
                nc.sync.dma_start(out=XT[:, c, :], in_=xTr[:, c, :])
            nc.sync.dma_start(out=BD, in_=bdfull)
            dma_pair_kconst(0)
            dma_pair_weights(1)
            dma_pair_kconst(1)
            nc.sync.dma_start(out=IDT, in_=ident)
            nc.vector.memset(VALL.rearrange("p h k c -> p (h k) c")[:, :, 64:65], 1.0)

            # ---------- production of one pair (generator of PE pieces) ----
            stq4 = STQB.rearrange("p c (t ww) -> p c t ww", ww=W)

            def gen_production(p, act_drains=False):
                """Yield rows_emitted after each small PE piece. With
                act_drains (prologue only, ACT otherwise idle) the even-half
                drains go to ACT to halve the lead-in drain latency."""
                wt = wts[p]
                qb, kb2 = biases[p]
                he, ho = 2 * p, 2 * p + 1

                def drain_even(out, in0, sc):
                    if act_drains:
                        nc.scalar.activation(out, in0, IDN, bias=sc, scale=1.0)
                    else:
                        nc.vector.tensor_scalar(out=out, in0=in0, scalar1=sc,
                                                scalar2=None, op0=ADD)

                def copy_even(out, in0):
                    if act_drains:
                        nc.scalar.copy(out, in0)
                    else:
                        nc.vector.tensor_copy(out, in0)
                # q chains (feature-major)
                for qh in range(NQH):
                    qsl = slice(qh * 512, (qh + 1) * 512)
                    ps = psA_p.tile([128, 512], F32, tag="ps1",
                                    name=f"psq{p}_{qh}")
                    for c in range(NC):
                        nc.tensor.matmul(ps, lhsT=wt[:, c, 0:128],
                                         rhs=XT[:, c, qsl],
                                         start=(c == 0), stop=(c == NC - 1))
                        yield 512
                    drain_even(QALL[0:64, he, qsl], ps[0:64], qb[0:64])
                    nc.vector.tensor_scalar(out=QALL[64:128, ho, qsl],
                                            in0=ps[64:128], scalar1=qb[64:128],
                                            scalar2=None, op0=ADD)
                    if act_drains:
                        nc.scalar.activation(STQB[:, p, qsl], ps, IDN,
                                             bias=qb, scale=1.0)
                    else:
                        nc.vector.tensor_scalar(out=STQB[:, p, qsl], in0=ps,
                                                scalar1=qb, scalar2=None,
                                                op0=ADD)
                    yield 0
                # k chains
                for qh in range(NQH):
                    qsl = slice(qh * 512, (qh + 1) * 512)
                    ps = psA_p.tile([128, 512], F32, tag="ps1",
                                    name=f"psk{p}_{qh}")
                    for c in range(NC):
                        nc.tensor.matmul(ps, lhsT=wt[:, c, 128:256],
                                         rhs=XT[:, c, qsl],
                                         start=(c == 0), stop=(c == NC - 1))
                        yield 512
                    drain_even(KALL[0:64, he, qsl], ps[0:64], kb2[0:64])
                    nc.vector.tensor_scalar(out=KALL[64:128, ho, qsl],
                                            in0=ps[64:128], scalar1=kb2[64:128],
                                            scalar2=None, op0=ADD)
                    yield 0
                # rel-pos: h then w, in two 16-row halves
                # in the prologue the attention "s" slots are idle; using
                # them for rel avoids ps1-WAR stalls against q/k drains
                rtag = "s" if act_drains else "ps1"
                for half in range(2):
                    hsl = slice(half * 512, (half + 1) * 512)
                    psr = psA_p.tile([128, 16, 32], F32, tag=rtag,
                                     name=f"psrh{p}_{half}")
                    for j in range(16):
                        h = half * 16 + j
                        nc.tensor.matmul(psr[:, j, :], lhsT=BD[:, h, :],
                                         rhs=STQB[:, p, h * 32:(h + 1) * 32],
                                         start=True, stop=True,
                                         skip_group_check=True)
                        if j % 4 == 3:
                            yield 128
                    copy_even(
                        QALL[64:96, he, hsl].rearrange("p (hb t) -> p hb t", t=32),
                        psr[64:96])
                    nc.vector.tensor_copy(
                        QALL[0:32, ho, hsl].rearrange("p (hb t) -> p hb t", t=32),
                        psr[0:32])
                    yield 0
                qvwE = QALL[96:128, he, :].rearrange("p (t wb) -> p wb t", wb=32)
                qvwO = QALL[32:64, ho, :].rearrange("p (t wb) -> p wb t", wb=32)
                for half in range(2):
                    psr = psA_p.tile([128, 16, 32], F32, tag=rtag,
                                     name=f"psrw{p}_{half}")
                    for j in range(16):
                        w = half * 16 + j
                        nc.tensor.matmul(psr[:, j, :], lhsT=BD[:, w, :],
                                         rhs=stq4[:, p, :, w],
                                         start=True, stop=True,
                                         skip_group_check=True)
                        if j % 4 == 3:
                            yield 128
                    copy_even(qvwE[:, half * 16:(half + 1) * 16, :],
                              psr[96:128])
                    nc.vector.tensor_copy(qvwO[:, half * 16:(half + 1) * 16, :],
                                          psr[32:64])
                    yield 0

                # v chains (token-major), 2 token-blocks per psum tile,
                # single ACT drain per tile. v-bias is folded into the proj
                # bias on the host (y is affine in v).
                for vb in range(4):
                    psv = psA_p.tile([128, 2, 128], F32, tag="ps1",
                                     name=f"psv{p}_{vb}")
                    for t2 in range(2):
                        tb = vb * 2 + t2
                        for c in range(NC):
                            nc.tensor.matmul(
                                psv[:, t2, :],
                                lhsT=XT[:, c, tb * 128:(tb + 1) * 128],
                                rhs=wt[:, c, 256:384],
                                start=(c == 0), stop=(c == NC - 1),
                                skip_group_check=True)
                            yield 128
                    if act_drains:
                        nc.scalar.copy(
                            VALL[:, he:ho + 1, vb * 2:(vb + 1) * 2, 0:64],
                            psv.rearrange("p t (hh f) -> p hh t f", hh=2))
                    else:
                        nc.vector.tensor_copy(
                            VALL[:, he:ho + 1, vb * 2:(vb + 1) * 2, 0:64],
                            psv.rearrange("p t (hh f) -> p hh t f", hh=2))
                    yield 0
            # ---------- attention pieces ----------
            def emit_qk_exp(h, kb):
                ps_s = psA_p.tile([128, N], F32, tag="s", name=f"s{h}_{kb}")
                for qh in range(NQH):
                    nc.tensor.matmul(
                        ps_s[:, qh * 512:(qh + 1) * 512],
                        lhsT=KALL[:, h, kb * 128:(kb + 1) * 128],
                        rhs=QALL[:, h, qh * 512:(qh + 1) * 512],
                        start=True, stop=True)
                pt = pt_p.tile([128, N], BF16, tag="pt", name=f"pt{h}_{kb}")
                nc.scalar.activation(pt, ps_s, EXP)
                return pt

            def emit_pv(h, pts, aodt):
                """PV in two 4-token-block halves; the softmax divide is
                fused into the drain: one reciprocal per half + one
                stride-0-broadcast multiply PSUM->aodt."""
                par = h % 2
                fsl = slice(par * 64, par * 64 + 64)
                for half in range(2):
                    pv = psB_p.tile([128, 4, 128], F32, tag="pv",
                                    name=f"pv{h}_{half}")
                    for t2 in range(4):
                        tb = half * 4 + t2
                        for kb in range(NKB):
                            nc.tensor.matmul(
                                pv[:, t2, 0:65],
                                lhsT=pts[kb][:, tb * 128:(tb + 1) * 128],
                                rhs=VALL[:, h, kb, :],
                                start=(kb == 0), stop=(kb == NKB - 1),
                                skip_group_check=True)
                    rh = stg_p.tile([128, 4], F32, tag="rd", bufs=4,
                                    name=f"rd{h}_{half}")
                    nc.vector.reciprocal(
                        rh, pv[:, :, 64:65].rearrange("p t one -> p (t one)"))
                    in1 = bass.AP(tensor=rh.tensor, offset=rh.offset,
                                  ap=[list(rh.ap[0]), [1, 4], [0, 64]])
                    nc.vector.tensor_mul(
                        aodt[:, half * 4:(half + 1) * 4, fsl],
                        pv[:, :, 0:64], in1)

            def emit_cleanup(p, aodt):
                for tp in range(4):
                    pst = psA_p.tile([128, 2, 128], BF16, tag="ps1",
                                     name=f"pst{p}_{tp}")
                    for t2 in range(2):
                        nc.tensor.transpose(pst[:, t2, :],
                                            aodt[:, tp * 2 + t2, :], IDT)
                    nc.vector.tensor_copy(
                        AOD[:, p, tp * 256:(tp + 1) * 256],
                        pst.rearrange("p a b -> p (a b)"))

            # ---------- the pipelined main loop ----------
            fillers = None      # generator producing pair p+1
            prev_pts = None     # PT tiles of previous head
            prev_aodt = None    # token-major attention-out of prev head's pair

            # prologue: produce pair 0 outright, then bridge the drain
            # window (QK(0,0) waits on DVE/ACT drains) with early pulls of
            # production(1) so the PE never idles nor drops its p-state.
            for _ in gen_production(0, act_drains=True):
                pass
            fillers = gen_production(1)
            bridge = 6144.0
            while fillers is not None and bridge > 0:
                try:
                    bridge -= next(fillers)
                except StopIteration:
                    fillers = None

            PAIR_ROWS = 20480.0  # PE rows per pair production

            aodts = {}
            for j in range(HEADS):
                h = j
                p = j // 2
                par = j % 2
                if par == 0:
                    aodts[p] = stg_p.tile([128, NKB, 128], BF16, tag="aodt",
                                          name=f"aodt{p}")
                    # DMAs for pair p+2 production (consumed via fillers at
                    # steps 2p+2, 2p+3)
                    if p + 2 < NC:
                        dma_pair_weights(p + 2)
                        biases[p + 2] = dma_pair_biases(p + 2)
                        dma_pair_kconst(p + 2)
                    if p == 2:
                        WP = cons_p.tile([128, NC, DIM], BF16)
                        for c in range(NC):
                            nc.sync.dma_start(
                                out=WP[:, c, :],
                                in_=wprojT[c * 128:(c + 1) * 128, :])
                        nc.sync.dma_start(
                            out=PBIAS, in_=projb.rearrange("(c p) -> p c", p=128))
                    # production of pair p+1 interleaves into this pair's steps
                    if fillers is None and p + 1 < NC:
                        fillers = gen_production(p + 1)

                # QK + exp for head h, pulling fillers to keep PE fed.
                # Front-load q/k/rel of the next pair into the even head's
                # slots so the next pair's QK is never production-gated;
                # only v (needed a step later) rides the odd head's slots.
                pts = []
                budget = 0.0
                for kb in range(NKB):
                    pts.append(emit_qk_exp(h, kb))
                    budget += (14336.0 if par == 0 else 6144.0) / 8.0
                    while fillers is not None and budget > 0:
                        try:
                            budget -= next(fillers)
                        except StopIteration:
                            fillers = None
                # cleanup of pair p-1: its last PV was emitted in the
                # previous step; deferring to after this step's QK loop gives
                # the DVE recip/divide chain a full QK window to complete
                # before the PE reaches the transposes.
                if par == 1 and p >= 1:
                    emit_cleanup(p - 1, aodts.pop(p - 1))

                # proj first-half (c=0..2 over AOD pairs 0-2, ready after
                # cleanup(2)) as late-step fillers where production runs dry
                if j >= 9 and j <= 11:
                    for ob4 in range(4):
                        ob, qh = divmod((j - 9) * 4 + ob4, NQH)
                        qsl = slice(qh * 512, (qh + 1) * 512)
                        ps = psA_p.tile([128, 512], F32, tag="ps1",
                                        name=f"psyA{ob}_{qh}")
                        for c in range(4):
                            nc.tensor.matmul(
                                ps, lhsT=WP[:, c, ob * 128:(ob + 1) * 128],
                                rhs=AOD[:, c, qsl],
                                start=(c == 0), stop=(c == 3))
                        nc.vector.tensor_copy(YA[:, ob, qsl], ps)

                # PV of the previous head
                if prev_pts is not None:
                    emit_pv(h - 1, prev_pts, prev_aodt)
                prev_pts, prev_aodt = pts, aodts[p]

                # drain any residual production at pair boundaries
                if par == 1 and fillers is not None:
                    for _ in fillers:
                        pass
                    fillers = None

            emit_pv(HEADS - 1, prev_pts, prev_aodt)
            emit_cleanup(NC - 1, aodts.pop(NC - 1))

        # ---------- proj + bias + out ----------
        with tc.tile_pool(name="ps4", bufs=6, space="PSUM") as ps4_p, \
             tc.tile_pool(name="wpp", bufs=1) as wp2_p:
            YSB = xt_p.tile([128, NC, N], F32, tag="xtslot")
            for ob in range(NC):
                for qh in range(NQH):
                    qsl = slice(qh * 512, (qh + 1) * 512)
                    ps = ps4_p.tile([128, 512], F32, tag="ps4",
                                    name=f"psp{ob}_{qh}")
                    for c in range(4, NC):
                        nc.tensor.matmul(
                            ps, lhsT=WP[:, c, ob * 128:(ob + 1) * 128],
                            rhs=AOD[:, c, qsl],
                            start=(c == 4), stop=(c == NC - 1))
                    nc.vector.scalar_tensor_tensor(
                        YSB[:, ob, qsl], ps, PBIAS[:, ob:ob + 1],
                        YA[:, ob, qsl], ADD, ADD)
                    nc.sync.dma_start(out=y[ob * 128:(ob + 1) * 128, qsl],
                                      in_=YSB[:, ob, qsl])
        aod_p.release()
        xt_p.release()
        cons_p.release()
        vall_p.release()
        kall_p.release()
        qall_p.release()

    nc.compile()
    return nc


def host_prep(x, qkv_w, qkv_b, proj_w, proj_b, rel_pos_h, rel_pos_w):
    """full inputs -> list of 8 per-core in_maps"""
    import ml_dtypes
    x = np.asarray(x, np.float32)
    qkv_w = np.asarray(qkv_w, np.float32)
    qkv_b = np.asarray(qkv_b, np.float32)
    proj_w = np.asarray(proj_w, np.float32)
    proj_b = np.asarray(proj_b, np.float32)
    rel_pos_h = np.asarray(rel_pos_h, np.float32)
    rel_pos_w = np.asarray(rel_pos_w, np.float32)

    wqkvT = np.ascontiguousarray(qkv_w.T).copy()   # (768, 2304)
    wqkvT[:, :DIM] *= SCALE
    qkvb2 = qkv_b.copy()
    qkvb2[:DIM] *= SCALE
    # packed (pair, 768, 384) = q_p | k_p | v_p
    wpack = np.empty((NC, DIM, 384), np.float32)
    for p in range(NC):
        wpack[p, :, 0:128] = wqkvT[:, p * 128:(p + 1) * 128]
        wpack[p, :, 128:256] = wqkvT[:, DIM + p * 128:DIM + (p + 1) * 128]
        wpack[p, :, 256:384] = wqkvT[:, 2 * DIM + p * 128:2 * DIM + (p + 1) * 128]
    wpack = wpack.astype(ml_dtypes.bfloat16)
    wprojT = np.ascontiguousarray(proj_w.T).astype(ml_dtypes.bfloat16)
    projb_f = proj_b + proj_w @ qkvb2[2 * DIM:]

    idx = np.arange(H)
    Rh = rel_pos_h[idx[:, None] - idx[None, :] + (H - 1)]  # (32,32,64) [q,k,c]
    Rw = rel_pos_w[idx[:, None] - idx[None, :] + (W - 1)]
    # block-diagonal rel stationary: rows 0:64 hold (0 | RhT | RwT) for
    # even heads, rows 64:128 hold (RhT | RwT | 0) for odd heads
    bdfull = np.zeros((128, H, 128), np.float32)
    bdfull[0:64, :, 64:96] = Rh.transpose(2, 0, 1) / SCALE
    bdfull[0:64, :, 96:128] = Rw.transpose(2, 0, 1) / SCALE
    bdfull[64:128, :, 0:32] = Rh.transpose(2, 0, 1) / SCALE
    bdfull[64:128, :, 32:64] = Rw.transpose(2, 0, 1) / SCALE
    bdfull = bdfull.astype(ml_dtypes.bfloat16)

    k = np.arange(N)
    kconst = np.zeros((64, N), np.float32)
    kconst[:32] = (k[None, :] // 32 == np.arange(32)[:, None])
    kconst[32:] = (k[None, :] % 32 == np.arange(32)[:, None])
    kconst = kconst.astype(ml_dtypes.bfloat16)

    ident = np.eye(128, dtype=ml_dtypes.bfloat16)

    shared = dict(wpack=wpack, qkvb=qkvb2, wprojT=wprojT, projb=projb_f,
                  bdfull=bdfull, kconst=kconst, ident=ident)
    in_maps = []
    for b in range(B):
        xTb = np.ascontiguousarray(x[b].reshape(N, DIM).T).astype(ml_dtypes.bfloat16)
        in_maps.append(dict(xT=xTb, **shared))
    return in_maps


def get_nc():
    if "nc" not in _CACHE:
        _CACHE["nc"] = build_nc()
    return _CACHE["nc"]


def kernel(**inputs):
    nc = get_nc()
    in_maps = host_prep(**inputs)
    res = bass_utils.run_bass_kernel_spmd(nc, in_maps, core_ids=list(range(NCORES)))
    out = np.stack([np.asarray(r["y"]).T for r in res.results], axis=0)
    return np.ascontiguousarray(out).reshape(B, H, W, DIM).astype(np.float32)


# revision 63
# speedup vs baseline: 1.0007x; 1.0007x over previous
"""Trainium2 Bass kernel: ViT-style global attention with decomposed
relative position bias (B=8, 32x32 tokens, dim 768, 12 heads, hd 64).

Sharding: data-parallel over batch B=8 -> one image per NeuronCore,
weights replicated, no collectives. All matmul operands bf16 (fp32
PSUM accumulate); TimelineSim ~160us/core vs 206us baseline.

Fully software-pipelined single pass over 6 head-pairs (emission order
defines each engine's program order; Tile adds the semaphores):
  - production(P) [bf16 chains over XT]: q/k features feature-major,
    rel-pos per pair via a shared block-diagonal stationary (the rel-h
    and rel-w placements occupy disjoint columns; garbage rows are
    never drained), V token-major. Emitted as PE "fillers" interleaved
    between the previous pair's QK matmuls with a front-loaded budget
    (q/rel/k complete during the even head so the next pair's QK is
    never production-gated).
  - attention(head): per kb ONE K=128 matmul gives scale*S^T+bias in
    PSUM (bias rides contraction rows 64:128 against indicator rows of
    K'); exp on ACT -> PT bf16. PV is token-major: lhsT = PT 128x128
    chunk (stationary), rhs = V[keys,65] (64 v-feats | ones) ->
    out[tok,65] accumulated over kb in two 4-token-block PSUM tiles;
    col 64 = softmax denominator for free. Drain fuses the divide: one
    DVE reciprocal per half + one stride-0-broadcast multiply into
    token-major aodt.
  - cleanup(pair): PE transposes (identity matmul) back to
    feature-major AOD bf16 for proj, deferred half a step so the DVE
    chain clears first.
  - proj: c=0..3 partial contractions run as late-step fillers into an
    SBUF accumulator YA (they only need AOD pairs 0..3); the tail is
    just c=4..5 plus a scalar_tensor_tensor drain (ps+bias+YA). v-bias
    is folded into proj_b on the host (y is affine in v); q-scale and
    rel-table scale are host-folded.
Engine totals: PE ~134us (bound), ACT ~108us (exp-dominated), DVE
~109us (all PSUM drains; GPSIMD cannot touch PSUM on TRN2), Pool idle.
Prologue rel tiles borrow the idle attention "s" PSUM slots.
"""

import os

import numpy as np

import concourse.bacc as bacc
import concourse.bass as bass
import concourse.tile as tile
from concourse import mybir
from concourse import bass_utils

B, H, W, DIM = 8, 32, 32, 768
HEADS, HD = 12, 64
N = H * W  # 1024
NCORES = 8
SCALE = HD ** -0.5
F32 = mybir.dt.float32
F32R = mybir.dt.float32r
BF16 = mybir.dt.bfloat16
EXP = mybir.ActivationFunctionType.Exp
IDN = mybir.ActivationFunctionType.Identity
ADD = mybir.AluOpType.add
MULT = mybir.AluOpType.mult

NC = DIM // 128      # 6 feature chunks == head pairs
NKB = N // 128       # 8 key/token blocks
NQH = N // 512       # 2 query halves

_CACHE = {}

NWARM = int(os.environ.get("K_WARM", "30"))
PT_BUFS = int(os.environ.get("K_PT", "16"))


def build_nc():
    nc = bacc.Bacc("TRN2", target_bir_lowering=False, debug=False)

    xT = nc.dram_tensor("xT", (DIM, N), BF16, kind="ExternalInput").ap()
    # packed per-pair weight columns: (pair, DIM, 384) = (q_p | k_p | v_p)
    wpack = nc.dram_tensor("wpack", (NC, DIM, 384), BF16, kind="ExternalInput").ap()
    qkvb = nc.dram_tensor("qkvb", (3 * DIM,), F32, kind="ExternalInput").ap()
    wprojT = nc.dram_tensor("wprojT", (DIM, DIM), BF16, kind="ExternalInput").ap()
    projb = nc.dram_tensor("projb", (DIM,), F32, kind="ExternalInput").ap()
    # bdfull: host-packed block-diagonal rel stationary [128, H, 128]
    bdfull = nc.dram_tensor("bdfull", (128, H, 128), BF16, kind="ExternalInput").ap()
    kconst = nc.dram_tensor("kconst", (64, N), BF16, kind="ExternalInput").ap()
    ident = nc.dram_tensor("ident", (128, 128), BF16, kind="ExternalInput").ap()
    y = nc.dram_tensor("y", (DIM, N), F32, kind="ExternalOutput").ap()

    qkvb2 = qkvb.rearrange("(c p one) -> c p one", p=128, one=1)   # [18][128,1]

    with tile.TileContext(nc) as tc:
        # ---- long-lived pools (bottom of SBUF stack) ----
        qall_p = tc.alloc_tile_pool(name="qall", bufs=1)
        kall_p = tc.alloc_tile_pool(name="kall", bufs=1)
        vall_p = tc.alloc_tile_pool(name="vall", bufs=1)
        cons_p = tc.alloc_tile_pool(name="cons", bufs=1)
        xt_p = tc.alloc_tile_pool(name="xtp", bufs=1)
        aod_p = tc.alloc_tile_pool(name="aod", bufs=1)

        QALL = qall_p.tile([128, HEADS, N], BF16)
        KALL = kall_p.tile([128, HEADS, N], BF16)
        VALL = vall_p.tile([128, HEADS, NKB, 65], BF16)
        STQB = cons_p.tile([128, NC, N], BF16)
        BD = cons_p.tile([128, H, 128], BF16)
        IDT = cons_p.tile([128, 128], BF16)
        PBIAS = cons_p.tile([128, NC], F32)
        YA = cons_p.tile([128, NC, N], F32)   # proj partial (c=0..2)
        XT = xt_p.tile([128, NC, N], BF16, tag="xtslot")
        AOD = aod_p.tile([128, NC, N], BF16)

        with tc.tile_pool(name="wpool", bufs=1) as w_p, \
             tc.tile_pool(name="bias", bufs=4) as b_p, \
             tc.tile_pool(name="pt", bufs=PT_BUFS) as pt_p, \
             tc.tile_pool(name="stg", bufs=2) as stg_p, \
             tc.tile_pool(name="psA", bufs=2, space="PSUM") as psA_p, \
             tc.tile_pool(name="psB", bufs=2, space="PSUM") as psB_p:

            # ---------- constant / early DMAs ----------
            wts = {}

            def dma_pair_weights(p):
                wt = w_p.tile([128, NC, 384], BF16, tag="wt", bufs=2,
                              name=f"wt{p}")
                nc.sync.dma_start(
                    out=wt, in_=wpack[p].rearrange("(c p2) f -> p2 c f", p2=128))
                wts[p] = wt

            def dma_pair_biases(p):
                qb = b_p.tile([128, 1], F32, tag="bias", bufs=6, name=f"qb{p}")
                nc.sync.dma_start(out=qb, in_=qkvb2[p])
                kb2 = b_p.tile([128, 1], F32, tag="bias", bufs=6, name=f"kb{p}")
                nc.sync.dma_start(out=kb2, in_=qkvb2[6 + p])
                return qb, kb2

            def dma_pair_kconst(p):
                nc.sync.dma_start(out=KALL[64:128, 2 * p, :], in_=kconst)
                nc.sync.dma_start(out=KALL[0:64, 2 * p + 1, :], in_=kconst)

            # PE p-state warm-up while DMAs land (memset first so DVE's later
            # big memsets don't gate the first warm matmul)
            if NWARM:
                jnk = cons_p.tile([128, 128], BF16)
                nc.vector.memset(jnk, 0.5)
                # preload the Exp table while the PE warms up so the first
                # real exp doesn't eat the 1.3us table load
                nc.scalar.activation(jnk[:, 0:8], jnk[:, 0:8], EXP)
                wps = psA_p.tile([128, 128], F32, tag="ps1", name="warmps")
                for _ in range(NWARM):
                    nc.tensor.matmul(wps, lhsT=jnk, rhs=jnk,
                                     start=True, stop=True,
                                     skip_group_check=True)

            dma_pair_weights(0)
            biases = {0: dma_pair_biases(0)}
            biases[1] = dma_pair_biases(1)
            xTr = xT.rearrange("(c p2) n -> p2 c n", p2=128)
            for c in range(NC):
                nc.sync.dma_start(out=XT[:, c, :], in_=xTr[:, c, :])
            nc.sync.dma_start(out=BD, in_=bdfull)
            dma_pair_kconst(0)
            dma_pair_weights(1)
            dma_pair_kconst(1)
            nc.sync.dma_start(out=IDT, in_=ident)
            nc.vector.memset(VALL.rearrange("p h k c -> p (h k) c")[:, :, 64:65], 1.0)

            # ---------- production of one pair (generator of PE pieces) ----
            stq4 = STQB.rearrange("p c (t ww) -> p c t ww", ww=W)

            def gen_production(p, act_drains=False):
                """Yield rows_emitted after each small PE piece. With
                act_drains (prologue only, ACT otherwise idle) the even-half
                drains go to ACT to halve the lead-in drain latency."""
                wt = wts[p]
                qb, kb2 = biases[p]
                he, ho = 2 * p, 2 * p + 1

                def drain_even(out, in0, sc):
                    if act_drains:
                        nc.scalar.activation(out, in0, IDN, bias=sc, scale=1.0)
                    else:
                        nc.vector.tensor_scalar(out=out, in0=in0, scalar1=sc,
                                                scalar2=None, op0=ADD)

                def copy_even(out, in0):
                    if act_drains:
                        nc.scalar.copy(out, in0)
                    else:
                        nc.vector.tensor_copy(out, in0)
                # q chains (feature-major)
                for qh in range(NQH):
                    qsl = slice(qh * 512, (qh + 1) * 512)
                    ps = psA_p.tile([128, 512], F32, tag="ps1",
                                    name=f"psq{p}_{qh}")
                    for c in range(NC):
                        nc.tensor.matmul(ps, lhsT=wt[:, c, 0:128],
                                         rhs=XT[:, c, qsl],
                                         start=(c == 0), stop=(c == NC - 1))
                        yield 512
                    drain_even(QALL[0:64, he, qsl], ps[0:64], qb[0:64])
                    nc.vector.tensor_scalar(out=QALL[64:128, ho, qsl],
                                            in0=ps[64:128], scalar1=qb[64:128],
                                            scalar2=None, op0=ADD)
                    if act_drains:
                        nc.scalar.activation(STQB[:, p, qsl], ps, IDN,
                                             bias=qb, scale=1.0)
                    else:
                        nc.vector.tensor_scalar(out=STQB[:, p, qsl], in0=ps,
                                                scalar1=qb, scalar2=None,
                                                op0=ADD)
                    yield 0
                # k chains
                for qh in range(NQH):
                    qsl = slice(qh * 512, (qh + 1) * 512)
                    ps = psA_p.tile([128, 512], F32, tag="ps1",
                                    name=f"psk{p}_{qh}")
                    for c in range(NC):
                        nc.tensor.matmul(ps, lhsT=wt[:, c, 128:256],
                                         rhs=XT[:, c, qsl],
                                         start=(c == 0), stop=(c == NC - 1))
                        yield 512
                    drain_even(KALL[0:64, he, qsl], ps[0:64], kb2[0:64])
                    nc.vector.tensor_scalar(out=KALL[64:128, ho, qsl],
                                            in0=ps[64:128], scalar1=kb2[64:128],
                                            scalar2=None, op0=ADD)
                    yield 0
                # rel-pos: h then w, in two 16-row halves
                # in the prologue the attention "s" slots are idle; using
                # them for rel avoids ps1-WAR stalls against q/k drains
                rtag = "s" if act_drains else "ps1"
                for half in range(2):
                    hsl = slice(half * 512, (half + 1) * 512)
                    psr = psA_p.tile([128, 16, 32], F32, tag=rtag,
                                     name=f"psrh{p}_{half}")
                    for j in range(16):
                        h = half * 16 + j
                        nc.tensor.matmul(psr[:, j, :], lhsT=BD[:, h, :],
                                         rhs=STQB[:, p, h * 32:(h + 1) * 32],
                                         start=True, stop=True,
                                         skip_group_check=True)
                        if j % 4 == 3:
                            yield 128
                    copy_even(
                        QALL[64:96, he, hsl].rearrange("p (hb t) -> p hb t", t=32),
                        psr[64:96])
                    nc.vector.tensor_copy(
                        QALL[0:32, ho, hsl].rearrange("p (hb t) -> p hb t", t=32),
                        psr[0:32])
                    yield 0
                qvwE = QALL[96:128, he, :].rearrange("p (t wb) -> p wb t", wb=32)
                qvwO = QALL[32:64, ho, :].rearrange("p (t wb) -> p wb t", wb=32)
                for half in range(2):
                    psr = psA_p.tile([128, 16, 32], F32, tag=rtag,
                                     name=f"psrw{p}_{half}")
                    for j in range(16):
                        w = half * 16 + j
                        nc.tensor.matmul(psr[:, j, :], lhsT=BD[:, w, :],
                                         rhs=stq4[:, p, :, w],
                                         start=True, stop=True,
                                         skip_group_check=True)
                        if j % 4 == 3:
                            yield 128
                    copy_even(qvwE[:, half * 16:(half + 1) * 16, :],
                              psr[96:128])
                    nc.vector.tensor_copy(qvwO[:, half * 16:(half + 1) * 16, :],
                                          psr[32:64])
                    yield 0

                # v chains (token-major), 2 token-blocks per psum tile,
                # single ACT drain per tile. v-bias is folded into the proj
                # bias on the host (y is affine in v).
                for vb in range(4):
                    psv = psA_p.tile([128, 2, 128], F32, tag="ps1",
                                     name=f"psv{p}_{vb}")
                    for t2 in range(2):
                        tb = vb * 2 + t2
                        for c in range(NC):
                            nc.tensor.matmul(
                                psv[:, t2, :],
                                lhsT=XT[:, c, tb * 128:(tb + 1) * 128],
                                rhs=wt[:, c, 256:384],
                                start=(c == 0), stop=(c == NC - 1),
                                skip_group_check=True)
                            yield 128
                    if act_drains:
                        nc.scalar.copy(
                            VALL[:, he:ho + 1, vb * 2:(vb + 1) * 2, 0:64],
                            psv.rearrange("p t (hh f) -> p hh t f", hh=2))
                    else:
                        nc.vector.tensor_copy(
                            VALL[:, he:ho + 1, vb * 2:(vb + 1) * 2, 0:64],
                            psv.rearrange("p t (hh f) -> p hh t f", hh=2))
                    yield 0
            # ---------- attention pieces ----------
            def emit_qk_exp(h, kb):
                ps_s = psA_p.tile([128, N], F32, tag="s", name=f"s{h}_{kb}")
                for qh in range(NQH):
                    nc.tensor.matmul(
                        ps_s[:, qh * 512:(qh + 1) * 512],
                        lhsT=KALL[:, h, kb * 128:(kb + 1) * 128],
                        rhs=QALL[:, h, qh * 512:(qh + 1) * 512],
                        start=True, stop=True)
                pt = pt_p.tile([128, N], BF16, tag="pt", name=f"pt{h}_{kb}")
                nc.scalar.activation(pt, ps_s, EXP)
                return pt

            def emit_pv(h, pts, aodt):
                """PV in two 4-token-block halves; the softmax divide is
                fused into the drain: one reciprocal per half + one
                stride-0-broadcast multiply PSUM->aodt."""
                par = h % 2
                fsl = slice(par * 64, par * 64 + 64)
                for half in range(2):
                    pv = psB_p.tile([128, 4, 128], F32, tag="pv",
                                    name=f"pv{h}_{half}")
                    for t2 in range(4):
                        tb = half * 4 + t2
                        for kb in range(NKB):
                            nc.tensor.matmul(
                                pv[:, t2, 0:65],
                                lhsT=pts[kb][:, tb * 128:(tb + 1) * 128],
                                rhs=VALL[:, h, kb, :],
                                start=(kb == 0), stop=(kb == NKB - 1),
                                skip_group_check=True)
                    rh = stg_p.tile([128, 4], F32, tag="rd", bufs=4,
                                    name=f"rd{h}_{half}")
                    nc.vector.reciprocal(
                        rh, pv[:, :, 64:65].rearrange("p t one -> p (t one)"))
                    in1 = bass.AP(tensor=rh.tensor, offset=rh.offset,
                                  ap=[list(rh.ap[0]), [1, 4], [0, 64]])
                    nc.vector.tensor_mul(
                        aodt[:, half * 4:(half + 1) * 4, fsl],
                        pv[:, :, 0:64], in1)

            def emit_cleanup(p, aodt):
                for tp in range(4):
                    pst = psA_p.tile([128, 2, 128], BF16, tag="ps1",
                                     name=f"pst{p}_{tp}")
                    for t2 in range(2):
                        nc.tensor.transpose(pst[:, t2, :],
                                            aodt[:, tp * 2 + t2, :], IDT)
                    nc.vector.tensor_copy(
                        AOD[:, p, tp * 256:(tp + 1) * 256],
                        pst.rearrange("p a b -> p (a b)"))

            # ---------- the pipelined main loop ----------
            fillers = None      # generator producing pair p+1
            prev_pts = None     # PT tiles of previous head
            prev_aodt = None    # token-major attention-out of prev head's pair

            # prologue: produce pair 0 outright, then bridge the drain
            # window (QK(0,0) waits on DVE/ACT drains) with early pulls of
            # production(1) so the PE never idles nor drops its p-state.
            for _ in gen_production(0, act_drains=True):
                pass
            fillers = gen_production(1)
            bridge = 6144.0
            while fillers is not None and bridge > 0:
                try:
                    bridge -= next(fillers)
                except StopIteration:
                    fillers = None

            PAIR_ROWS = 20480.0  # PE rows per pair production

            aodts = {}
            ya_items = []

            def make_ya(ob, qh):
                def emit():
                    qsl = slice(qh * 512, (qh + 1) * 512)
                    ps = psA_p.tile([128, 512], F32, tag="ps1",
                                    name=f"psyA{ob}_{qh}")
                    for c in range(4):
                        nc.tensor.matmul(
                            ps, lhsT=WP[:, c, ob * 128:(ob + 1) * 128],
                            rhs=AOD[:, c, qsl],
                            start=(c == 0), stop=(c == 3))
                    nc.vector.tensor_copy(YA[:, ob, qsl], ps)
                return emit

            for j in range(HEADS):
                h = j
                p = j // 2
                par = j % 2
                if par == 0:
                    aodts[p] = stg_p.tile([128, NKB, 128], BF16, tag="aodt",
                                          name=f"aodt{p}")
                    # DMAs for pair p+2 production (consumed via fillers at
                    # steps 2p+2, 2p+3)
                    if p + 2 < NC:
                        dma_pair_weights(p + 2)
                        biases[p + 2] = dma_pair_biases(p + 2)
                        dma_pair_kconst(p + 2)
                    if p == 2:
                        WP = cons_p.tile([128, NC, DIM], BF16)
                        for c in range(NC):
                            nc.sync.dma_start(
                                out=WP[:, c, :],
                                in_=wprojT[c * 128:(c + 1) * 128, :])
                        nc.sync.dma_start(
                            out=PBIAS, in_=projb.rearrange("(c p) -> p c", p=128))
                    # production of pair p+1 interleaves into this pair's steps
                    if fillers is None and p + 1 < NC:
                        fillers = gen_production(p + 1)

                # QK + exp for head h, pulling fillers to keep PE fed.
                # Front-load q/k/rel of the next pair into the even head's
                # slots so the next pair's QK is never production-gated;
                # only v (needed a step later) rides the odd head's slots.
                if j == 9:
                    ya_items.extend(make_ya(ob, qh)
                                    for ob in range(NC) for qh in range(NQH))
                pts = []
                budget = 0.0
                for kb in range(NKB):
                    pts.append(emit_qk_exp(h, kb))
                    budget += (14336.0 if par == 0 else 6144.0) / 8.0
                    while fillers is not None and budget > 0:
                        try:
                            budget -= next(fillers)
                        except StopIteration:
                            fillers = None
                    # proj first-half chains (c=0..3 over AOD pairs 0..3,
                    # ready after cleanup(3)) fill the slots once production
                    # runs dry in the last pairs
                    if fillers is None and ya_items and kb % 2 == 1:
                        ya_items.pop(0)()
                # cleanup of pair p-1: its last PV was emitted in the
                # previous step; deferring to after this step's QK loop gives
                # the DVE recip/divide chain a full QK window to complete
                # before the PE reaches the transposes.
                if par == 1 and p >= 1:
                    emit_cleanup(p - 1, aodts.pop(p - 1))

                # drain any proj first-half chains not yet emitted
                if j == 11:
                    while ya_items:
                        ya_items.pop(0)()

                # PV of the previous head
                if prev_pts is not None:
                    emit_pv(h - 1, prev_pts, prev_aodt)
                prev_pts, prev_aodt = pts, aodts[p]

                # drain any residual production at pair boundaries
                if par == 1 and fillers is not None:
                    for _ in fillers:
                        pass
                    fillers = None

            emit_pv(HEADS - 1, prev_pts, prev_aodt)
            emit_cleanup(NC - 1, aodts.pop(NC - 1))

        # ---------- proj + bias + out ----------
        with tc.tile_pool(name="ps4", bufs=6, space="PSUM") as ps4_p, \
             tc.tile_pool(name="wpp", bufs=1) as wp2_p:
            YSB = xt_p.tile([128, NC, N], F32, tag="xtslot")
            for ob in range(NC):
                for qh in range(NQH):
                    qsl = slice(qh * 512, (qh + 1) * 512)
                    ps = ps4_p.tile([128, 512], F32, tag="ps4",
                                    name=f"psp{ob}_{qh}")
                    for c in range(4, NC):
                        nc.tensor.matmul(
                            ps, lhsT=WP[:, c, ob * 128:(ob + 1) * 128],
                            rhs=AOD[:, c, qsl],
                            start=(c == 4), stop=(c == NC - 1))
                    nc.vector.scalar_tensor_tensor(
                        YSB[:, ob, qsl], ps, PBIAS[:, ob:ob + 1],
                        YA[:, ob, qsl], ADD, ADD)
                    nc.sync.dma_start(out=y[ob * 128:(ob + 1) * 128, qsl],
                                      in_=YSB[:, ob, qsl])
        aod_p.release()
        xt_p.release()
        cons_p.release()
        vall_p.release()
        kall_p.release()
        qall_p.release()

    nc.compile()
    return nc


def host_prep(x, qkv_w, qkv_b, proj_w, proj_b, rel_pos_h, rel_pos_w):
    """full inputs -> list of 8 per-core in_maps"""
    import ml_dtypes
    x = np.asarray(x, np.float32)
    qkv_w = np.asarray(qkv_w, np.float32)
    qkv_b = np.asarray(qkv_b, np.float32)
    proj_w = np.asarray(proj_w, np.float32)
    proj_b = np.asarray(proj_b, np.float32)
    rel_pos_h = np.asarray(rel_pos_h, np.float32)
    rel_pos_w = np.asarray(rel_pos_w, np.float32)

    wqkvT = np.ascontiguousarray(qkv_w.T).copy()   # (768, 2304)
    wqkvT[:, :DIM] *= SCALE
    qkvb2 = qkv_b.copy()
    qkvb2[:DIM] *= SCALE
    # packed (pair, 768, 384) = q_p | k_p | v_p
    wpack = np.empty((NC, DIM, 384), np.float32)
    for p in range(NC):
        wpack[p, :, 0:128] = wqkvT[:, p * 128:(p + 1) * 128]
        wpack[p, :, 128:256] = wqkvT[:, DIM + p * 128:DIM + (p + 1) * 128]
        wpack[p, :, 256:384] = wqkvT[:, 2 * DIM + p * 128:2 * DIM + (p + 1) * 128]
    wpack = wpack.astype(ml_dtypes.bfloat16)
    wprojT = np.ascontiguousarray(proj_w.T).astype(ml_dtypes.bfloat16)
    projb_f = proj_b + proj_w @ qkvb2[2 * DIM:]

    idx = np.arange(H)
    Rh = rel_pos_h[idx[:, None] - idx[None, :] + (H - 1)]  # (32,32,64) [q,k,c]
    Rw = rel_pos_w[idx[:, None] - idx[None, :] + (W - 1)]
    # block-diagonal rel stationary: rows 0:64 hold (0 | RhT | RwT) for
    # even heads, rows 64:128 hold (RhT | RwT | 0) for odd heads
    bdfull = np.zeros((128, H, 128), np.float32)
    bdfull[0:64, :, 64:96] = Rh.transpose(2, 0, 1) / SCALE
    bdfull[0:64, :, 96:128] = Rw.transpose(2, 0, 1) / SCALE
    bdfull[64:128, :, 0:32] = Rh.transpose(2, 0, 1) / SCALE
    bdfull[64:128, :, 32:64] = Rw.transpose(2, 0, 1) / SCALE
    bdfull = bdfull.astype(ml_dtypes.bfloat16)

    k = np.arange(N)
    kconst = np.zeros((64, N), np.float32)
    kconst[:32] = (k[None, :] // 32 == np.arange(32)[:, None])
    kconst[32:] = (k[None, :] % 32 == np.arange(32)[:, None])
    kconst = kconst.astype(ml_dtypes.bfloat16)

    ident = np.eye(128, dtype=ml_dtypes.bfloat16)

    shared = dict(wpack=wpack, qkvb=qkvb2, wprojT=wprojT, projb=projb_f,
                  bdfull=bdfull, kconst=kconst, ident=ident)
    in_maps = []
    for b in range(B):
        xTb = np.ascontiguousarray(x[b].reshape(N, DIM).T).astype(ml_dtypes.bfloat16)
        in_maps.append(dict(xT=xTb, **shared))
    return in_maps


def get_nc():
    if "nc" not in _CACHE:
        _CACHE["nc"] = build_nc()
    return _CACHE["nc"]


def kernel(**inputs):
    nc = get_nc()
    in_maps = host_prep(**inputs)
    res = bass_utils.run_bass_kernel_spmd(nc, in_maps, core_ids=list(range(NCORES)))
    out = np.stack([np.asarray(r["y"]).T for r in res.results], axis=0)
    return np.ascontiguousarray(out).reshape(B, H, W, DIM).astype(np.float32)
